# revision 1
# baseline (speedup 1.0000x reference)
"""GNN message-passing kernel for 8 TRN2 NeuronCores (Bass/Tile).

Sharding: graphs partitioned across cores (32/core; batch sorted -> contiguous
node ranges). Edges routed to dst-owner core, laid out as padded
(section=src-core-pair, window=dst_local//128) cells so the SPMD instruction
schedule is core-independent. Per-edge h gathered from AllGathered node tables
via custom dma_gather (int16 idx into 2-core-pair sub-tables). Segment-sum by
dst via PE matmuls against bf16 selection matrices generated on DVE
(is_equal vs iota). BN stats via small AllReduce; f32 master h in DRAM with a
bf16 working copy in SBUF. Attention pooling core-local; 4x256 target rows
assembled via a small AllReduce.
"""
import numpy as np
import ml_dtypes
import concourse.bass as bass
import concourse.mybir as mybir
import concourse.bacc as bacc
import concourse.tile as tile
from concourse import library_config
from concourse.masks import make_identity
from concourse.bass_utils import run_bass_kernel_spmd

F32 = mybir.dt.float32
BF16 = mybir.dt.bfloat16
I16 = mybir.dt.int16
I32 = mybir.dt.int32
AF = mybir.ActivationFunctionType
ALU = mybir.AluOpType
BF = ml_dtypes.bfloat16

NCORES = 8
DIM = 64
HEAD = 8
DH = 8
NUM = 4
N = 100_000
B = 256
GPC = B // NCORES
NSEC = 4
CHW = 4  # windows per chunk


def wrap16(idx, cols):
    n = idx.shape[0]
    assert n % 16 == 0 and n // 16 == cols
    w = np.zeros((16, cols), np.int16)
    w[np.arange(n) % 16, np.arange(n) // 16] = idx.astype(np.int16)
    return np.tile(w, (8, 1))


def preprocess(inputs):
    batch = np.asarray(inputs["batch"])
    node_core = (batch // GPC).astype(np.int64)
    n0 = np.searchsorted(batch, np.arange(NCORES) * GPC)
    n1 = np.append(n0[1:], N)
    Nc = (n1 - n0).astype(np.int64)
    NT = int(np.ceil((Nc.max() + 1) / 512) * 512)
    NW = NT // 128
    NCH = (NW + CHW - 1) // CHW

    loc = np.arange(N) - n0[node_core]
    grow = node_core * NT + loc

    sched = {"NT": NT, "NW": NW, "NCH": NCH, "Nc": Nc, "n0": n0}
    percore = [dict() for _ in range(NCORES)]

    for c in range(NCORES):
        m = np.zeros(NT, np.float32)
        m[: Nc[c]] = 1.0
        percore[c]["mask"] = np.broadcast_to(m, (64, NT)).copy()

    for bi, key in ((0, "edge_index1"), (1, "edge_index2")):
        ei = np.asarray(inputs[key])
        src, dst = ei[0].astype(np.int64), ei[1].astype(np.int64)
        dcore = node_core[dst]
        sec = node_core[src] // 2
        w = loc[dst] // 128
        rel = loc[dst] - w * 128
        lidx = grow[src] - sec * 2 * NT

        counts = np.zeros((NCORES, NSEC, NW), np.int64)
        np.add.at(counts, (dcore, sec, w), 1)
        CW = np.ceil(counts.max(axis=0) / 128).astype(np.int64)
        NB = int(CW.sum())
        blk_w = []
        cell_off = np.zeros((NSEC, NW), np.int64)
        secchunk = np.zeros((NSEC, NCH + 1), np.int64)
        off = 0
        for s in range(NSEC):
            for ch in range(NCH):
                secchunk[s, ch] = off
                for ww in range(ch * CHW, min((ch + 1) * CHW, NW)):
                    cell_off[s, ww] = off
                    blk_w += [ww] * int(CW[s, ww])
                    off += int(CW[s, ww])
            secchunk[s, NCH] = off
        assert off == NB
        fsec = np.full(NW, -1, np.int64)
        lsec = np.full(NW, -1, np.int64)
        for ww in range(NW):
            secs = [s for s in range(NSEC) if CW[s, ww] > 0]
            if secs:
                fsec[ww], lsec[ww] = secs[0], secs[-1]
        sched[f"CW{bi}"] = CW
        sched[f"NB{bi}"] = NB
        sched[f"blk_w{bi}"] = np.array(blk_w, np.int64)
        sched[f"secchunk{bi}"] = secchunk
        sched[f"fsec{bi}"] = fsec
        sched[f"lsec{bi}"] = lsec
        sched[f"cell_off{bi}"] = cell_off

        eattr = np.asarray(inputs["edge_attr1"]) if bi == 0 else None
        for c in range(NCORES):
            emask = dcore == c
            es, ew, erel, elidx = sec[emask], w[emask], rel[emask], lidx[emask]
            order = np.lexsort((erel, ew, es))
            es, ew, erel, elidx = es[order], ew[order], erel[order], elidx[order]
            cellid = es * NW + ew
            _, cnts = np.unique(cellid, return_counts=True)
            ranks = (np.concatenate([np.arange(k) for k in cnts])
                     if len(cnts) else np.zeros(0, np.int64))
            slot = cell_off[es, ew] * 128 + ranks
            gidx = np.zeros(NB * 128, np.int16)
            grel = np.full(NB * 128, -100.0, np.float32)
            gidx[slot] = elidx.astype(np.int16)
            grel[slot] = erel
            percore[c][f"gidx{bi}"] = wrap16(gidx, NB * 8)
            percore[c][f"rel{bi}"] = np.ascontiguousarray(
                grel.reshape(NB, 128).T).astype(BF)
            if bi == 0:
                ea = np.zeros((NB * 128, DIM), np.float32)
                ea[slot] = eattr[emask][order]
                percore[c]["attr"] = np.ascontiguousarray(
                    ea.reshape(NB, 128, DIM).transpose(1, 0, 2).reshape(128, NB * DIM))

    for bi, (xk, ek) in enumerate((("x1_nodes", "emb1"), ("x2_nodes", "emb2"))):
        et = np.zeros((128, DIM), np.float32)
        et[:100] = np.asarray(inputs[ek])
        x = np.asarray(inputs[xk])
        for c in range(NCORES):
            xl = np.full(NT, 100, np.int16)
            xl[: Nc[c]] = x[n0[c]:n1[c]]
            percore[c][f"xidx{bi}"] = wrap16(xl, NT // 16)
            percore[c][f"embt{bi}"] = et

    dirn = np.asarray(inputs["direction"])[:, 0]
    t1 = np.asarray(inputs["target1_index"]).astype(np.int64)
    t2 = np.asarray(inputs["target2_index"]).astype(np.int64)
    dir2 = np.asarray(inputs["direction2"])[:, 0]
    for c in range(NCORES):
        g = np.zeros((GPC, NT), np.float32)
        gl = batch[n0[c]:n1[c]] - c * GPC
        g[gl, np.arange(Nc[c])] = 1.0
        percore[c]["G"] = g.astype(BF)
        percore[c]["GT"] = np.ascontiguousarray(
            g.reshape(GPC, NT // 128, 128).transpose(2, 1, 0)
            .reshape(128, (NT // 128) * GPC)).astype(BF)
        d = np.zeros(NT, np.float32)
        d[: Nc[c]] = dirn[n0[c]:n1[c]]
        percore[c]["dirn"] = d[None, :].astype(BF)
        gsl = slice(c * GPC, (c + 1) * GPC)
        percore[c]["dirt"] = np.concatenate(
            [dirn[t1[gsl]], dirn[t2[gsl]]])[None, :].astype(BF)
        percore[c]["dir2"] = dir2[gsl][None, :].astype(np.float32)

    for c in range(NCORES):
        for bi in range(2):
            offs = np.zeros((128, 1), np.int32)
            slots = np.full(128, 1024, np.int16)
            k = 0
            for tset, tarr in ((0, t1), (1, t2)):
                for g in range(B):
                    node = int(tarr[g])
                    if node_core[node] == c:
                        offs[k, 0] = loc[node]
                        slots[k] = bi * 512 + tset * 256 + g
                        k += 1
            assert k <= 128, f"target cap exceeded: {k}"
            percore[c][f"tsrc{bi}"] = offs
            percore[c][f"tslot{bi}"] = wrap16(slots, 8)
            o = np.zeros((128, 1), np.int32)
            for tset in range(2):
                for j in range(GPC):
                    o[tset * GPC + j, 0] = bi * 512 + tset * 256 + c * GPC + j
            percore[c][f"koff{bi}"] = o

    shared = {}
    for bi, p in ((0, "1"), (1, "2")):
        w1 = np.asarray(inputs[f"mlp{p}_w1"])
        w2 = np.asarray(inputs[f"mlp{p}_w2"])
        for l in range(NUM):
            shared[f"w1_{bi}_{l}"] = w1[l].astype(BF)
            shared[f"w2_{bi}_{l}"] = w2[l].astype(BF)
        shared[f"b1_{bi}"] = np.asarray(inputs[f"mlp{p}_b1"]).T.copy()
        shared[f"b2_{bi}"] = np.asarray(inputs[f"mlp{p}_b2"]).T.copy()
        shared[f"gam_{bi}"] = np.asarray(inputs[f"bn{p}_gamma"]).T.copy()
        shared[f"bet_{bi}"] = np.asarray(inputs[f"bn{p}_beta"]).T.copy()
        for nm in ("Wq", "Wk", "Wv"):
            shared[f"{nm}_{bi}"] = np.ascontiguousarray(
                np.asarray(inputs[f"{nm}{p}"]).transpose(1, 0, 2)
                .reshape(DIM + 1, DIM)).astype(BF)
    shared["fc11_w"] = np.asarray(inputs["fc11_w"])
    shared["fc11_b"] = np.asarray(inputs["fc11_b"])[:, None]
    shared["fc12_w"] = np.asarray(inputs["fc12_w"])
    shared["fc12_b"] = np.asarray(inputs["fc12_b"])[:, None]
    shared["bd8"] = (np.kron(np.eye(HEAD, dtype=np.float32),
                             np.ones((DH, 1), np.float32)) / np.sqrt(DH))
    shared["e8"] = np.kron(np.eye(HEAD, dtype=np.float32),
                           np.ones((1, DH), np.float32))
    shared["iota"] = np.broadcast_to(
        np.arange(128, dtype=np.float32), (128, 128)).astype(BF).copy()
    for c in range(NCORES):
        percore[c].update(shared)
    return sched, percore


def bap(t_ap, offset_elems, ap_list):
    return bass.AP(t_ap.tensor, t_ap.offset + offset_elems, ap_list)


def build(sched):
    NT, NW, NCH = sched["NT"], sched["NW"], sched["NCH"]
    NXC = NT // 128
    nc = bacc.Bacc(None, target_bir_lowering=False)

    def P(name, shape, dt=F32):
        return nc.declare_dram_parameter(name, list(shape), dt, isOutput=False)

    NB = [sched["NB0"], sched["NB1"]]
    ins = {}
    for bi in range(2):
        ins[f"gidx{bi}"] = P(f"gidx{bi}", [128, NB[bi] * 8], I16)
        ins[f"rel{bi}"] = P(f"rel{bi}", [128, NB[bi]], BF16)
        ins[f"xidx{bi}"] = P(f"xidx{bi}", [128, NT // 16], I16)
        ins[f"embt{bi}"] = P(f"embt{bi}", [128, DIM])
        ins[f"tsrc{bi}"] = P(f"tsrc{bi}", [128, 1], I32)
        ins[f"tslot{bi}"] = P(f"tslot{bi}", [128, 8], I16)
        ins[f"koff{bi}"] = P(f"koff{bi}", [128, 1], I32)
        for nm in ("Wq", "Wk", "Wv"):
            ins[f"{nm}_{bi}"] = P(f"{nm}_{bi}", [DIM + 1, DIM], BF16)
        for l in range(NUM):
            ins[f"w1_{bi}_{l}"] = P(f"w1_{bi}_{l}", [DIM, DIM], BF16)
            ins[f"w2_{bi}_{l}"] = P(f"w2_{bi}_{l}", [DIM, DIM], BF16)
        for nm in ("b1", "b2", "gam", "bet"):
            ins[f"{nm}_{bi}"] = P(f"{nm}_{bi}", [DIM, NUM])
    ins["attr"] = P("attr", [128, NB[0] * DIM])
    ins["mask"] = P("mask", [64, NT])
    ins["G"] = P("G", [GPC, NT], BF16)
    ins["GT"] = P("GT", [128, NXC * GPC], BF16)
    ins["dirn"] = P("dirn", [1, NT], BF16)
    ins["dirt"] = P("dirt", [1, 2 * GPC], BF16)
    ins["dir2"] = P("dir2", [1, GPC])
    ins["fc11_w"] = P("fc11_w", [DIM + 1, 32])
    ins["fc11_b"] = P("fc11_b", [32, 1])
    ins["fc12_w"] = P("fc12_w", [32, 1])
    ins["fc12_b"] = P("fc12_b", [1, 1])
    ins["bd8"] = P("bd8", [64, HEAD])
    ins["e8"] = P("e8", [HEAD, 64])
    ins["iota"] = P("iota", [128, 128], BF16)

    out_ext = nc.declare_dram_parameter("out", [1, GPC], F32, isOutput=True)

    hb = [nc.dram_tensor(f"h{bi}b", [NT, DIM], F32) for bi in range(2)]
    hTd = [nc.dram_tensor(f"h{bi}Td", [64, NT], F32) for bi in range(2)]
    table = [nc.dram_tensor(f"table{bi}", [NCORES * NT, DIM], F32,
                            addr_space="Shared") for bi in range(2)]
    stats_in = nc.dram_tensor("stats_in", [64, 2], F32)
    stats_out = nc.dram_tensor("stats_out", [64, 2], F32, addr_space="Shared")
    TROWS = 1024 + 16
    tb_in = nc.dram_tensor("tb_in", [TROWS, DIM], F32)
    tb_out = nc.dram_tensor("tb_out", [TROWS, DIM], F32, addr_space="Shared")
    RG = [list(range(NCORES))]

    blk_w = [sched["blk_w0"], sched["blk_w1"]]
    secchunk = [sched["secchunk0"], sched["secchunk1"]]
    CWb = [sched["CW0"], sched["CW1"]]
    fsec = [sched["fsec0"], sched["fsec1"]]
    lsec = [sched["lsec0"], sched["lsec1"]]
    cell_off = [sched["cell_off0"], sched["cell_off1"]]

    with tile.TileContext(nc) as tc:
        nc.gpsimd.load_library(library_config.mlp)
        with (
            tc.tile_pool(name="res", bufs=1) as res,
            tc.tile_pool(name="big", bufs=2) as big,
            tc.tile_pool(name="selp", bufs=1) as selp,
            tc.tile_pool(name="sm", bufs=1) as sm,
            tc.tile_pool(name="smd", bufs=2) as smd,
            tc.tile_pool(name="ps", bufs=2, space="PSUM") as ps,
            tc.tile_pool(name="pse", bufs=1, space="PSUM") as pse,
        ):
            ident = res.tile([128, 128], F32, tag="ident")
            make_identity(nc, ident[:])
            ident_b = res.tile([128, 128], BF16, tag="identb")
            nc.vector.tensor_copy(ident_b[:], ident[:])
            iota_t = res.tile([128, 128], BF16, tag="iota")
            nc.sync.dma_start(iota_t[:], ins["iota"][:])
            rel_t = [res.tile([128, NB[bi]], BF16, tag=f"rel{bi}",
                              name=f"rel_t{bi}") for bi in range(2)]
            for bi in range(2):
                nc.sync.dma_start(rel_t[bi][:], ins[f"rel{bi}"][:])

            hT = [res.tile([64, NT], BF16, tag=f"hT{bi}", name=f"hT{bi}")
                  for bi in range(2)]
            xT = res.tile([64, NT], BF16, tag="xT")
            ebuf = res.tile([HEAD, NT], BF16, tag="e1")
            GTt = res.tile([128, NXC * GPC], BF16, tag="GT")
            nc.sync.dma_start(GTt[:], ins["GT"][:])
            osum = res.tile([64, GPC], F32, tag="osum")

            zt = sm.tile([128, 512], F32, tag="zt")
            nc.vector.memset(zt[:], 0.0)
            for bi in range(2):
                flat = hb[bi].ap().rearrange("(a b) e -> a (b e)", a=128)
                wtot = flat.shape[1]
                o = 0
                while o < wtot:
                    wd = min(512, wtot - o)
                    nc.sync.dma_start(flat[:, o:o + wd], zt[:, :wd])
                    o += wd

            for bi in range(2):
                xi = sm.tile([128, NT // 16], I16, tag="xi")
                nc.sync.dma_start(xi[:], ins[f"xidx{bi}"][:])
                QH = NXC // 4
                for q in range(4):
                    h0c = big.tile([128, QH * DIM], F32, tag="gt")
                    h03 = h0c[:].rearrange("p (n e) -> p n e", e=DIM)
                    nc.gpsimd.dma_gather(
                        h03, ins[f"embt{bi}"][:],
                        xi[:, q * (QH * 8):(q + 1) * (QH * 8)],
                        QH * 128, QH * 128, DIM, single_packet=False)
                    nc.sync.dma_start(
                        hb[bi].ap().rearrange("(n p) e -> p n e", p=128)
                        [:, q * QH:(q + 1) * QH, :], h03)
                    for xx in range(QH):
                        x = q * QH + xx
                        pt = pse.tile([64, 128], F32, tag="misc", name=f"tp_{bi}_{x}")
                        nc.tensor.transpose(pt[:],
                                            h0c[:, xx * DIM:(xx + 1) * DIM],
                                            ident[:])
                        hx = smd.tile([64, 128], F32, tag="hx")
                        nc.scalar.activation(hx[:], pt[:], AF.Copy)
                        nc.sync.dma_start(
                            hTd[bi].ap()[:, x * 128:(x + 1) * 128], hx[:])
                        nc.vector.tensor_copy(
                            hT[bi][:, x * 128:(x + 1) * 128], hx[:])
            for bi in range(2):
                nc.gpsimd.collective_compute(
                    "AllGather", ALU.bypass, replica_groups=RG,
                    ins=[hb[bi].ap().opt()], outs=[table[bi].ap().opt()])

            wt = {}
            for bi in range(2):
                for l in range(NUM):
                    for nm in ("w1", "w2"):
                        t = res.tile([DIM, DIM], BF16, tag=f"{nm}{bi}{l}",
                                     name=f"{nm}_{bi}_{l}t")
                        nc.sync.dma_start(t[:], ins[f"{nm}_{bi}_{l}"][:])
                        wt[f"{nm}{bi}{l}"] = t
                for nm in ("b1", "b2", "gam", "bet"):
                    t = res.tile([DIM, NUM], F32, tag=f"{nm}{bi}",
                                 name=f"{nm}_{bi}t")
                    nc.sync.dma_start(t[:], ins[f"{nm}_{bi}"][:])
                    wt[f"{nm}{bi}"] = t

            # ================= main layers =================
            for l in range(NUM):
                for bi in range(2):
                    for ch in range(NCH):
                        w_lo, w_hi = ch * CHW, min((ch + 1) * CHW, NW)
                        hc = smd.tile([64, CHW * 128], F32, tag="hc")
                        nc.sync.dma_start(hc[:, :(w_hi - w_lo) * 128],
                                          hTd[bi].ap()[:, w_lo * 128:w_hi * 128])
                        msgs = {}
                        for s_ in range(NSEC):
                            b0 = int(secchunk[bi][s_, ch])
                            b1_ = int(secchunk[bi][s_, ch + 1])
                            nb = b1_ - b0
                            if nb == 0:
                                continue
                            gi = smd.tile([128, nb * 8], I16, tag="gi")
                            nc.sync.dma_start(gi[:],
                                              ins[f"gidx{bi}"][:, b0 * 8:b1_ * 8])
                            gt = big.tile([128, nb * DIM], F32, tag="gt")
                            gt3 = gt[:].rearrange("p (n e) -> p n e", e=DIM)
                            nc.gpsimd.dma_gather(
                                gt3, table[bi][s_ * 2 * NT:(s_ + 1) * 2 * NT, :],
                                gi[:], nb * 128, nb * 128, DIM,
                                single_packet=False)
                            msg = big.tile([128, nb * DIM], BF16, bufs=1,
                                           tag=f"msg{s_}", name=f"msg{s_}_t")
                            if bi == 0:
                                at = big.tile([128, nb * DIM], F32, tag="at")
                                nc.sync.dma_start(
                                    at[:], ins["attr"][:, b0 * DIM:b1_ * DIM])
                                nc.vector.tensor_tensor(
                                    out=msg[:], in0=gt[:], in1=at[:], op=ALU.add)
                                nc.vector.tensor_scalar_max(msg[:], msg[:], 0.0)
                            else:
                                nc.vector.tensor_copy(msg[:], gt[:])
                            msgs[s_] = (msg, b0)
                        for w in range(w_lo, w_hi):
                            sl = slice(w * 128, (w + 1) * 128)
                            hsl = slice((w - w_lo) * 128, (w - w_lo + 1) * 128)
                            if CWb[bi][:, w].sum() == 0:
                                nc.vector.tensor_copy(xT[:, sl], hc[:, hsl])
                                continue
                            pw = ps.tile([64, 128], F32, tag="pw", bufs=2,
                                         name=f"pw_{l}_{bi}_{w}")
                            started = False
                            for s_ in range(NSEC):
                                cw = int(CWb[bi][s_, w])
                                if cw == 0:
                                    continue
                                msg, b0 = msgs[s_]
                                msg3 = msg[:].rearrange("p (n e) -> p n e", e=DIM)
                                c0 = int(cell_off[bi][s_, w])
                                sel = selp.tile([128, cw * 128], BF16, tag="sel", bufs=2)
                                sel3 = sel[:].rearrange("p (n e) -> p n e", e=128)
                                io_ap = bap(iota_t[:], 0,
                                            [[128, 128], [0, cw], [1, 128]])
                                rl_ap = bap(rel_t[bi][:], c0,
                                            [[NB[bi], 128], [1, cw], [0, 128]])
                                nc.vector.tensor_tensor(
                                    out=sel3[:], in0=io_ap, in1=rl_ap,
                                    op=ALU.is_equal)
                                for j in range(cw):
                                    nc.tensor.matmul(
                                        pw[:], lhsT=msg3[:, c0 - b0 + j, :],
                                        rhs=sel3[:, j, :],
                                        start=(not started),
                                        stop=(s_ == lsec[bi][w] and j == cw - 1))
                                    started = True
                            nc.vector.tensor_tensor(
                                out=xT[:, sl], in0=pw[:], in1=hc[:, hsl],
                                op=ALU.add)

                    b1c = wt[f"b1{bi}"][:, l:l + 1]
                    b2c = wt[f"b2{bi}"][:, l:l + 1]
                    nmc = NT // 512

                    def mlp_chunk(j0):
                        p1 = ps.tile([64, 512], F32, tag="p1", bufs=1)
                        nc.tensor.matmul(p1[:], lhsT=wt[f"w1{bi}{l}"][:],
                                         rhs=xT[:, j0:j0 + 512], start=True,
                                         stop=True)
                        t1 = smd.tile([64, 512], BF16, tag="t1")
                        nc.scalar.activation(t1[:], p1[:], AF.Relu, bias=b1c)
                        p2 = ps.tile([64, 512], F32, tag="p2", bufs=1)
                        nc.tensor.matmul(p2[:], lhsT=wt[f"w2{bi}{l}"][:],
                                         rhs=t1[:], start=True, stop=True)
                        tm = smd.tile([64, 512], F32, tag="tm")
                        nc.scalar.activation(tm[:], p2[:], AF.Identity, bias=b2c)
                        mk = smd.tile([64, 512], F32, tag="mk")
                        nc.sync.dma_start(mk[:], ins["mask"][:, j0:j0 + 512])
                        nc.vector.tensor_tensor(out=tm[:], in0=tm[:], in1=mk[:],
                                                op=ALU.mult)
                        return tm, mk

                    parts = sm.tile([64, 2 * nmc], F32, tag="parts")
                    for jc in range(nmc):
                        tm, _ = mlp_chunk(jc * 512)
                        nc.vector.reduce_sum(parts[:, jc:jc + 1], tm[:],
                                             axis=mybir.AxisListType.X)
                        sq = smd.tile([64, 512], F32, tag="sq")
                        nc.vector.tensor_tensor(out=sq[:], in0=tm[:], in1=tm[:],
                                                op=ALU.mult)
                        nc.vector.reduce_sum(parts[:, nmc + jc:nmc + jc + 1],
                                             sq[:], axis=mybir.AxisListType.X)
                    st = sm.tile([64, 2], F32, tag="st")
                    nc.vector.reduce_sum(st[:, 0:1], parts[:, :nmc],
                                         axis=mybir.AxisListType.X)
                    nc.vector.reduce_sum(st[:, 1:2], parts[:, nmc:2 * nmc],
                                         axis=mybir.AxisListType.X)
                    nc.sync.dma_start(stats_in.ap()[:, :], st[:])
                    nc.gpsimd.collective_compute(
                        "AllReduce", ALU.add, replica_groups=RG,
                        ins=[stats_in.ap().opt()], outs=[stats_out.ap().opt()])
                    sts = sm.tile([64, 2], F32, tag="sts")
                    nc.sync.dma_start(sts[:], stats_out.ap()[:, :])
                    mv = sm.tile([64, 8], F32, tag="mv")
                    nc.vector.tensor_scalar_mul(mv[:, 0:1], sts[:, 0:1], 1.0 / N)
                    nc.vector.tensor_scalar_mul(mv[:, 1:2], sts[:, 1:2], 1.0 / N)
                    nc.vector.tensor_tensor(out=mv[:, 2:3], in0=mv[:, 0:1],
                                            in1=mv[:, 0:1], op=ALU.mult)
                    nc.vector.tensor_tensor(out=mv[:, 3:4], in0=mv[:, 1:2],
                                            in1=mv[:, 2:3], op=ALU.subtract)
                    nc.vector.tensor_scalar_add(mv[:, 3:4], mv[:, 3:4], 1e-5)
                    nc.scalar.activation(mv[:, 4:5], mv[:, 3:4], AF.Sqrt)
                    nc.vector.reciprocal(mv[:, 5:6], mv[:, 4:5])
                    nc.vector.tensor_tensor(out=mv[:, 6:7],
                                            in0=wt[f"gam{bi}"][:, l:l + 1],
                                            in1=mv[:, 5:6], op=ALU.mult)
                    nc.vector.tensor_tensor(out=mv[:, 7:8], in0=mv[:, 0:1],
                                            in1=mv[:, 6:7], op=ALU.mult)
                    nc.vector.tensor_tensor(out=mv[:, 7:8],
                                            in0=wt[f"bet{bi}"][:, l:l + 1],
                                            in1=mv[:, 7:8], op=ALU.subtract)
                    for jc in range(nmc):
                        j0 = jc * 512
                        tm, mk = mlp_chunk(j0)
                        hc2 = smd.tile([64, 512], F32, tag="hc2")
                        nc.sync.dma_start(hc2[:], hTd[bi].ap()[:, j0:j0 + 512])
                        sc1 = smd.tile([64, 512], F32, tag="sc1")
                        nc.scalar.activation(sc1[:], tm[:], AF.Copy,
                                             scale=mv[:, 6:7])
                        nc.vector.tensor_tensor(out=hc2[:], in0=hc2[:],
                                                in1=sc1[:], op=ALU.add)
                        nc.scalar.activation(sc1[:], mk[:], AF.Copy,
                                             scale=mv[:, 7:8])
                        nc.vector.tensor_tensor(out=hc2[:], in0=hc2[:],
                                                in1=sc1[:], op=ALU.add)
                        nc.sync.dma_start(hTd[bi].ap()[:, j0:j0 + 512], hc2[:])
                        nc.vector.tensor_copy(hT[bi][:, j0:j0 + 512], hc2[:])
                        hnm4 = smd.tile([128, 4 * DIM], F32, tag="hnm4")
                        for xx in range(4):
                            pt2 = pse.tile([128, 64], F32, tag="misc",
                                           name=f"tp2_{l}_{bi}_{jc}_{xx}")
                            nc.tensor.transpose(
                                pt2[:], hc2[:, xx * 128:(xx + 1) * 128],
                                ident[:64, :64])
                            nc.scalar.activation(
                                hnm4[:, xx * DIM:(xx + 1) * DIM], pt2[:],
                                AF.Copy)
                        nc.sync.dma_start(
                            hb[bi].ap().rearrange("(n p) e -> p n e", p=128)
                            [:, jc * 4:jc * 4 + 4, :],
                            hnm4[:].rearrange("p (n e) -> p n e", e=DIM))
                if l < NUM - 1:
                    for bi in range(2):
                        nc.gpsimd.collective_compute(
                            "AllGather", ALU.bypass, replica_groups=RG,
                            ins=[hb[bi].ap().opt()], outs=[table[bi].ap().opt()])

            # ================= target rows =================
            for r0 in range(0, TROWS, 128):
                rr = min(128, TROWS - r0)
                nc.sync.dma_start(tb_in.ap()[r0:r0 + rr, :], zt[:rr, :DIM])
            for bi in range(2):
                toff = sm.tile([128, 1], I32, tag="toff", name=f"toff{bi}")
                nc.sync.dma_start(toff[:], ins[f"tsrc{bi}"][:])
                trows = sm.tile([128, DIM], F32, tag="trows", name=f"trows{bi}")
                nc.gpsimd.indirect_dma_start(
                    out=trows[:], out_offset=None, in_=hb[bi].ap(),
                    in_offset=bass.IndirectOffsetOnAxis(ap=toff[:, 0:1], axis=0))
                tsl = sm.tile([128, 8], I16, tag="tsl", name=f"tsl{bi}")
                nc.sync.dma_start(tsl[:], ins[f"tslot{bi}"][:])
                nc.gpsimd.dma_scatter_add(
                    tb_in.ap()[:, :],
                    trows[:].rearrange("p (n e) -> p n e", e=DIM),
                    tsl[:], 128, 128, DIM, single_packet=False)
            nc.gpsimd.collective_compute(
                "AllReduce", ALU.add, replica_groups=RG,
                ins=[tb_in.ap().opt()], outs=[tb_out.ap().opt()])

            # ================= attention =================
            dirt_t = sm.tile([1, 2 * GPC], BF16, tag="dirt")
            nc.sync.dma_start(dirt_t[:], ins["dirt"][:])
            bd = sm.tile([64, HEAD], F32, tag="bd")
            nc.sync.dma_start(bd[:], ins["bd8"][:])
            e8t = sm.tile([HEAD, 64], F32, tag="e8t")
            nc.sync.dma_start(e8t[:], ins["e8"][:])

            for bi in range(2):
                wq = sm.tile([DIM, DIM], BF16, tag="wq", name=f"wq{bi}")
                nc.sync.dma_start(wq[:], ins[f"Wq_{bi}"][:DIM, :])
                wk = sm.tile([DIM, DIM], BF16, tag="wk", name=f"wk{bi}")
                nc.sync.dma_start(wk[:], ins[f"Wk_{bi}"][:DIM, :])
                wv = sm.tile([DIM, DIM], BF16, tag="wv", name=f"wv{bi}")
                nc.sync.dma_start(wv[:], ins[f"Wv_{bi}"][:DIM, :])
                wq64 = sm.tile([1, DIM], BF16, tag="wq64", name=f"wq64{bi}")
                nc.sync.dma_start(wq64[:], ins[f"Wq_{bi}"][DIM:DIM + 1, :])
                wk64 = sm.tile([1, DIM], BF16, tag="wk64", name=f"wk64{bi}")
                nc.sync.dma_start(wk64[:], ins[f"Wk_{bi}"][DIM:DIM + 1, :])
                wv64 = sm.tile([1, DIM], BF16, tag="wv64", name=f"wv64{bi}")
                nc.sync.dma_start(wv64[:], ins[f"Wv_{bi}"][DIM:DIM + 1, :])
                ko = sm.tile([128, 1], I32, tag="ko", name=f"ko{bi}")
                nc.sync.dma_start(ko[:], ins[f"koff{bi}"][:])
                krows = sm.tile([128, DIM], F32, tag="krows", name=f"krows{bi}")
                nc.gpsimd.indirect_dma_start(
                    out=krows[:], out_offset=None, in_=tb_out.ap(),
                    in_offset=bass.IndirectOffsetOnAxis(ap=ko[:, 0:1], axis=0))
                kp = pse.tile([64, 128], F32, tag="misc", name=f"kp{bi}")
                nc.tensor.transpose(kp[:], krows[:], ident[:])
                kft = sm.tile([64, 2 * GPC], BF16, tag="kft", name=f"kft{bi}")
                nc.vector.tensor_copy(kft[:], kp[:, :2 * GPC])
                ksp = pse.tile([64, 2 * GPC], F32, tag="misc2", name=f"ksp{bi}")
                nc.tensor.matmul(ksp[:], lhsT=wk[:], rhs=kft[:], start=True,
                                 stop=False)
                nc.tensor.matmul(ksp[:], lhsT=wk64[:], rhs=dirt_t[:],
                                 start=False, stop=True)
                kT = sm.tile([64, 2 * GPC], F32, tag="kT", name=f"kT{bi}")
                nc.vector.tensor_copy(kT[:], ksp[:])
                knms = []
                for tset in range(2):
                    knm_p = pse.tile([GPC, 64], F32, tag="misc",
                                     name=f"knmp_{bi}_{tset}")
                    nc.tensor.transpose(knm_p[:],
                                        kT[:, tset * GPC:(tset + 1) * GPC],
                                        ident[:64, :64])
                    knm1 = sm.tile([GPC, 64], BF16, tag=f"knmb{tset}",
                                   name=f"knm_{bi}_{tset}")
                    nc.vector.tensor_copy(knm1[:], knm_p[:])
                    knms.append(knm1)

                op_ = pse.tile([64, GPC], F32, tag="acc2", name=f"op{bi}")
                for tset in range(2):
                    for j0 in range(0, NT, 512):
                        sl = slice(j0, j0 + 512)
                        dnc = smd.tile([1, 512], BF16, tag="dnc")
                        nc.sync.dma_start(dnc[:], ins["dirn"][:, sl])
                        qp = ps.tile([64, 512], F32, tag="p1", bufs=1)
                        nc.tensor.matmul(qp[:], lhsT=wq[:], rhs=hT[bi][:, sl],
                                         start=True, stop=False)
                        nc.tensor.matmul(qp[:], lhsT=wq64[:], rhs=dnc[:],
                                         start=False, stop=True)
                        gch = smd.tile([GPC, 512], BF16, tag="gch")
                        nc.sync.dma_start(gch[:], ins["G"][:, sl])
                        kgp = pse.tile([64, 512], F32, tag="misc", name=f"kgp_{bi}_{tset}_{j0}")
                        nc.tensor.matmul(kgp[:], lhsT=knms[tset][:], rhs=gch[:],
                                         start=True, stop=True)
                        kgs = smd.tile([64, 512], F32, tag="kgs")
                        nc.vector.tensor_copy(kgs[:], kgp[:])
                        qk = smd.tile([64, 512], F32, tag="qk")
                        nc.vector.tensor_tensor(out=qk[:], in0=qp[:],
                                                in1=kgs[:], op=ALU.mult)
                        sp = pse.tile([HEAD, 512], F32, tag="misc2", name=f"sp_{bi}_{tset}_{j0}")
                        nc.tensor.matmul(sp[:], lhsT=bd[:], rhs=qk[:],
                                         start=True, stop=True)
                        nc.scalar.activation(ebuf[:, sl], sp[:], AF.Exp)
                    dp = pse.tile([GPC, HEAD], F32, tag="acc",
                                  name=f"dp_{bi}_{tset}")
                    for x in range(NXC):
                        ep = pse.tile([128, HEAD], BF16, tag="misc2",
                                      name=f"ep_{bi}_{tset}_{x}")
                        nc.tensor.transpose(ep[:],
                                            ebuf[:, x * 128:(x + 1) * 128],
                                            ident_b[:HEAD, :HEAD])
                        eT = smd.tile([128, HEAD], BF16, tag="eT")
                        nc.vector.tensor_copy(eT[:], ep[:])
                        nc.tensor.matmul(dp[:],
                                         lhsT=GTt[:, x * GPC:(x + 1) * GPC],
                                         rhs=eT[:], start=(x == 0),
                                         stop=(x == NXC - 1))
                    dt = sm.tile([GPC, HEAD], F32, tag="d0",
                                 name=f"dt_{bi}_{tset}")
                    nc.vector.tensor_copy(dt[:], dp[:])
                    nc.vector.tensor_scalar_add(dt[:], dt[:], 1e-16)
                    nc.vector.reciprocal(dt[:], dt[:])
                    dtb = sm.tile([GPC, HEAD], BF16, tag="db0",
                                  name=f"dtb_{bi}_{tset}")
                    nc.vector.tensor_copy(dtb[:], dt[:])
                    for x in range(NXC):
                        sl = slice(x * 128, (x + 1) * 128)
                        gc2 = smd.tile([GPC, 128], BF16, tag="gc2")
                        nc.sync.dma_start(gc2[:], ins["G"][:, sl])
                        rg1 = pse.tile([HEAD, 128], F32, tag="misc", name=f"rg1_{bi}_{tset}_{x}")
                        nc.tensor.matmul(rg1[:], lhsT=dtb[:], rhs=gc2[:],
                                         start=True, stop=True)
                        rgs1 = smd.tile([HEAD, 128], F32, tag="rgs1")
                        nc.vector.tensor_copy(rgs1[:], rg1[:])
                        wch = smd.tile([HEAD, 128], F32, tag="wch")
                        nc.vector.tensor_tensor(out=wch[:], in0=ebuf[:, sl],
                                                in1=rgs1[:], op=ALU.mult)
                        w64p = pse.tile([64, 128], F32, tag="misc2", name=f"w64_{bi}_{tset}_{x}")
                        nc.tensor.matmul(w64p[:], lhsT=e8t[:], rhs=wch[:],
                                         start=True, stop=True)
                        dnc2 = smd.tile([1, 128], BF16, tag="dnc2")
                        nc.sync.dma_start(dnc2[:], ins["dirn"][:, sl])
                        vp2 = pse.tile([64, 128], F32, tag="misc", name=f"vp2_{bi}_{tset}_{x}")
                        nc.tensor.matmul(vp2[:], lhsT=wv[:], rhs=hT[bi][:, sl],
                                         start=True, stop=False)
                        nc.tensor.matmul(vp2[:], lhsT=wv64[:], rhs=dnc2[:],
                                         start=False, stop=True)
                        vch = smd.tile([64, 128], F32, tag="vch")
                        nc.vector.tensor_copy(vch[:], vp2[:])
                        wv_ = smd.tile([64, 128], F32, tag="wv_")
                        nc.vector.tensor_tensor(out=wv_[:], in0=w64p[:],
                                                in1=vch[:], op=ALU.mult)
                        wvp = pse.tile([128, 64], F32, tag="misc", name=f"wvp_{bi}_{tset}_{x}")
                        nc.tensor.transpose(wvp[:], wv_[:], ident[:64, :64])
                        wvnm = smd.tile([128, 64], BF16, tag="wvnm")
                        nc.vector.tensor_copy(wvnm[:], wvp[:])
                        nc.tensor.matmul(op_[:], lhsT=wvnm[:],
                                         rhs=GTt[:, x * GPC:(x + 1) * GPC],
                                         start=(tset == 0 and x == 0),
                                         stop=(tset == 1 and x == NXC - 1))
                if bi == 0:
                    nc.vector.tensor_copy(osum[:], op_[:])
                else:
                    nc.vector.tensor_tensor(out=osum[:], in0=osum[:],
                                            in1=op_[:], op=ALU.add)

            fw1 = sm.tile([DIM, 32], F32, tag="fw1")
            nc.sync.dma_start(fw1[:], ins["fc11_w"][:DIM, :])
            fw1r = sm.tile([1, 32], F32, tag="fw1r")
            nc.sync.dma_start(fw1r[:], ins["fc11_w"][DIM:DIM + 1, :])
            fb1 = sm.tile([32, 1], F32, tag="fb1")
            nc.sync.dma_start(fb1[:], ins["fc11_b"][:])
            fw2 = sm.tile([32, 1], F32, tag="fw2")
            nc.sync.dma_start(fw2[:], ins["fc12_w"][:])
            fb2 = sm.tile([1, 1], F32, tag="fb2")
            nc.sync.dma_start(fb2[:], ins["fc12_b"][:])
            dir2_t = sm.tile([1, GPC], F32, tag="dir2")
            nc.sync.dma_start(dir2_t[:], ins["dir2"][:])
            zp = pse.tile([32, GPC], F32, tag="misc", name="zp_t")
            nc.tensor.matmul(zp[:], lhsT=fw1[:], rhs=osum[:], start=True,
                             stop=False)
            nc.tensor.matmul(zp[:], lhsT=fw1r[:], rhs=dir2_t[:], start=False,
                             stop=True)
            zs = sm.tile([32, GPC], F32, tag="zs")
            nc.scalar.activation(zs[:], zp[:], AF.Gelu, bias=fb1[:, 0:1])
            op2 = pse.tile([1, GPC], F32, tag="misc2", name="op2_t")
            nc.tensor.matmul(op2[:], lhsT=fw2[:], rhs=zs[:], start=True,
                             stop=True)
            fin = sm.tile([1, GPC], F32, tag="fin")
            nc.scalar.activation(fin[:], op2[:], AF.Sigmoid, bias=fb2[:, 0:1])
            nc.sync.dma_start(out_ext.ap()[:, :], fin[:])

    nc.compile()
    return nc


def kernel(**inputs):
    sched, percore = preprocess(inputs)
    nc = build(sched)
    res = run_bass_kernel_spmd(nc, percore, list(range(NCORES)))
    outs = [res.results[c]["out"][0] for c in range(NCORES)]
    return np.concatenate(outs).astype(np.float32)[:, None]


def kernel_timed(n_iter=8, **inputs):
    """kernel() plus a wall-clock estimate of per-NEFF execution time using
    pipelined repeated executions of the compiled executable."""
    import time
    import jax
    from jax.sharding import Mesh, PartitionSpec
    from jax.experimental.shard_map import shard_map
    from concourse import bass2jax

    sched, percore = preprocess(inputs)
    nc = build(sched)

    bass2jax.install_neuronx_cc_hook()
    partition_name = nc.partition_id_tensor.name if nc.partition_id_tensor else None
    in_names, out_names, out_avals, zero_outs = [], [], [], []
    for alloc in nc.m.functions[0].allocations:
        if not isinstance(alloc, mybir.MemoryLocationSet):
            continue
        name = alloc.memorylocations[0].name
        if alloc.kind == "ExternalInput":
            if name != partition_name:
                in_names.append(name)
        elif alloc.kind == "ExternalOutput":
            out_names.append(name)
            shape = tuple(alloc.tensor_shape)
            dtype = mybir.dt.np(alloc.dtype)
            out_avals.append(jax.core.ShapedArray(shape, dtype))
            zero_outs.append(np.zeros(shape, dtype))
    n_params = len(in_names)
    n_outs = len(out_avals)
    in_names.extend(out_names)
    if partition_name is not None:
        in_names.append(partition_name)

    def _body(*args):
        operands = list(args)
        if partition_name is not None:
            operands.append(bass2jax.partition_id_tensor())
        outs = bass2jax._bass_exec_p.bind(
            *operands, out_avals=tuple(out_avals), in_names=tuple(in_names),
            out_names=tuple(out_names), lowering_input_output_aliases=(),
            sim_require_finite=True, sim_require_nnan=True, nc=nc)
        return tuple(outs)

    devices = jax.devices()[:NCORES]
    mesh = Mesh(np.asarray(devices), ("core",))
    in_specs = (PartitionSpec("core"),) * (n_params + n_outs)
    out_specs = (PartitionSpec("core"),) * len(out_names)
    sharded = jax.jit(shard_map(_body, mesh=mesh, in_specs=in_specs,
                                out_specs=out_specs, check_rep=False),
                      keep_unused=True)
    per_core = [[np.asarray(percore[c][name]) for name in in_names[:n_params]]
                for c in range(NCORES)]
    concat_in = [np.concatenate([per_core[c][i] for c in range(NCORES)], axis=0)
                 for i in range(n_params)]
    concat_zeros = [np.zeros((NCORES * z.shape[0], *z.shape[1:]), z.dtype)
                    for z in zero_outs]
    args = [jax.device_put(a) for a in concat_in + concat_zeros]
    out = sharded(*args)
    jax.block_until_ready(out)
    # warm single call for dispatch-overhead baseline, then pipelined batch
    t0 = time.perf_counter()
    outs = [sharded(*args) for _ in range(n_iter)]
    jax.block_until_ready(outs)
    per_call = (time.perf_counter() - t0) / n_iter
    oidx = out_names.index("out")
    res = np.asarray(outs[-1][oidx]).reshape(NCORES, 1, GPC)
    full = np.concatenate([res[c][0] for c in range(NCORES)]).astype(np.float32)[:, None]
    return full, per_call



# revision 10
# speedup vs baseline: 1.7138x; 1.7138x over previous
"""GNN message-passing kernel for 8 TRN2 NeuronCores (Bass/Tile).

Sharding: graphs partitioned across cores (32/core; batch sorted -> contiguous
node ranges). Edges routed to dst-owner core, laid out as padded
(section=src-core-pair, window=dst_local//128) cells so the SPMD instruction
schedule is core-independent. Per-edge h gathered from AllGathered node tables
via custom dma_gather (int16 idx into 2-core-pair sub-tables). Segment-sum by
dst via PE matmuls against bf16 selection matrices generated on DVE
(is_equal vs iota). BN stats via small AllReduce overlapped with the other
branch's compute; f32 master h in DRAM. MLP intermediate t cached in SBUF
(bf16) so the BN apply needs no recompute. Attention pooling core-local with a
node-major apply pass; 4x256 target rows assembled via a small AllReduce.
"""
import numpy as np
import ml_dtypes
import concourse.bass as bass
import concourse.mybir as mybir
import concourse.bacc as bacc
import concourse.tile as tile
from concourse import library_config
from concourse.masks import make_identity
from concourse.bass_utils import run_bass_kernel_spmd

F32 = mybir.dt.float32
BF16 = mybir.dt.bfloat16
I16 = mybir.dt.int16
I32 = mybir.dt.int32
AF = mybir.ActivationFunctionType
ALU = mybir.AluOpType
BF = ml_dtypes.bfloat16

NCORES = 8
DIM = 64
HEAD = 8
DH = 8
NUM = 4
N = 100_000
B = 256
GPC = B // NCORES
NSEC = 4
CHW = 6  # windows per chunk


def wrap16(idx, cols):
    n = idx.shape[0]
    assert n % 16 == 0 and n // 16 == cols
    w = np.zeros((16, cols), np.int16)
    w[np.arange(n) % 16, np.arange(n) // 16] = idx.astype(np.int16)
    return np.tile(w, (8, 1))


def preprocess(inputs):
    batch = np.asarray(inputs["batch"])
    node_core = (batch // GPC).astype(np.int64)
    n0 = np.searchsorted(batch, np.arange(NCORES) * GPC)
    n1 = np.append(n0[1:], N)
    Nc = (n1 - n0).astype(np.int64)
    NT = int(np.ceil((Nc.max() + 1) / 512) * 512)
    NW = NT // 128
    NCH = (NW + CHW - 1) // CHW

    loc = np.arange(N) - n0[node_core]
    grow = node_core * NT + loc

    sched = {"NT": NT, "NW": NW, "NCH": NCH, "Nc": Nc, "n0": n0}
    percore = [dict() for _ in range(NCORES)]

    for c in range(NCORES):
        m = np.zeros(NT, np.float32)
        m[: Nc[c]] = 1.0
        percore[c]["mask"] = np.broadcast_to(m, (64, NT)).astype(BF).copy()

    for bi, key in ((0, "edge_index1"), (1, "edge_index2")):
        ei = np.asarray(inputs[key])
        src, dst = ei[0].astype(np.int64), ei[1].astype(np.int64)
        dcore = node_core[dst]
        sec = node_core[src] // 2
        w = loc[dst] // 128
        rel = loc[dst] - w * 128
        lidx = grow[src] - sec * 2 * NT

        counts = np.zeros((NCORES, NSEC, NW), np.int64)
        np.add.at(counts, (dcore, sec, w), 1)
        CW = np.ceil(counts.max(axis=0) / 128).astype(np.int64)
        NB = int(CW.sum())
        blk_w = []
        cell_off = np.zeros((NSEC, NW), np.int64)
        secchunk = np.zeros((NSEC, NCH + 1), np.int64)
        off = 0
        for s in range(NSEC):
            for ch in range(NCH):
                secchunk[s, ch] = off
                for ww in range(ch * CHW, min((ch + 1) * CHW, NW)):
                    cell_off[s, ww] = off
                    blk_w += [ww] * int(CW[s, ww])
                    off += int(CW[s, ww])
            secchunk[s, NCH] = off
        assert off == NB
        fsec = np.full(NW, -1, np.int64)
        lsec = np.full(NW, -1, np.int64)
        for ww in range(NW):
            secs = [s for s in range(NSEC) if CW[s, ww] > 0]
            if secs:
                fsec[ww], lsec[ww] = secs[0], secs[-1]
        sched[f"CW{bi}"] = CW
        sched[f"NB{bi}"] = NB
        sched[f"blk_w{bi}"] = np.array(blk_w, np.int64)
        sched[f"secchunk{bi}"] = secchunk
        sched[f"fsec{bi}"] = fsec
        sched[f"lsec{bi}"] = lsec
        sched[f"cell_off{bi}"] = cell_off

        eattr = np.asarray(inputs["edge_attr1"]) if bi == 0 else None
        for c in range(NCORES):
            emask = dcore == c
            es, ew, erel, elidx = sec[emask], w[emask], rel[emask], lidx[emask]
            order = np.lexsort((erel, ew, es))
            es, ew, erel, elidx = es[order], ew[order], erel[order], elidx[order]
            cellid = es * NW + ew
            _, cnts = np.unique(cellid, return_counts=True)
            ranks = (np.concatenate([np.arange(k) for k in cnts])
                     if len(cnts) else np.zeros(0, np.int64))
            slot = cell_off[es, ew] * 128 + ranks
            gidx = np.zeros(NB * 128, np.int16)
            grel = np.full(NB * 128, -100.0, np.float32)
            gidx[slot] = elidx.astype(np.int16)
            grel[slot] = erel
            percore[c][f"gidx{bi}"] = wrap16(gidx, NB * 8)
            percore[c][f"rel{bi}"] = np.ascontiguousarray(
                grel.reshape(NB, 128).T).astype(BF)
            if bi == 0:
                ea = np.zeros((NB * 128, DIM), np.float32)
                ea[slot] = eattr[emask][order]
                percore[c]["attr"] = np.ascontiguousarray(
                    ea.reshape(NB, 128, DIM).transpose(1, 0, 2)
                    .reshape(128, NB * DIM)).astype(BF)

    for bi, (xk, ek) in enumerate((("x1_nodes", "emb1"), ("x2_nodes", "emb2"))):
        et = np.zeros((128, DIM), np.float32)
        et[:100] = np.asarray(inputs[ek])
        x = np.asarray(inputs[xk])
        for c in range(NCORES):
            xl = np.full(NT, 100, np.int16)
            xl[: Nc[c]] = x[n0[c]:n1[c]]
            percore[c][f"xidx{bi}"] = wrap16(xl, NT // 16)
            percore[c][f"embt{bi}"] = et

    dirn = np.asarray(inputs["direction"])[:, 0]
    t1 = np.asarray(inputs["target1_index"]).astype(np.int64)
    t2 = np.asarray(inputs["target2_index"]).astype(np.int64)
    dir2 = np.asarray(inputs["direction2"])[:, 0]
    for c in range(NCORES):
        g = np.zeros((GPC, NT), np.float32)
        gl = batch[n0[c]:n1[c]] - c * GPC
        g[gl, np.arange(Nc[c])] = 1.0
        percore[c]["G"] = g.astype(BF)
        percore[c]["GT"] = np.ascontiguousarray(
            g.reshape(GPC, NT // 128, 128).transpose(2, 1, 0)
            .reshape(128, (NT // 128) * GPC)).astype(BF)
        d = np.zeros(NT, np.float32)
        d[: Nc[c]] = dirn[n0[c]:n1[c]]
        percore[c]["dirn"] = d[None, :].astype(BF)
        gsl = slice(c * GPC, (c + 1) * GPC)
        percore[c]["dirt"] = np.concatenate(
            [dirn[t1[gsl]], dirn[t2[gsl]]])[None, :].astype(BF)
        percore[c]["dir2"] = dir2[gsl][None, :].astype(np.float32)

    for c in range(NCORES):
        for bi in range(2):
            offs = np.zeros((128, 1), np.int32)
            slots = np.full(128, 1024, np.int16)
            k = 0
            for tset, tarr in ((0, t1), (1, t2)):
                for g in range(B):
                    node = int(tarr[g])
                    if node_core[node] == c:
                        offs[k, 0] = loc[node]
                        slots[k] = bi * 512 + tset * 256 + g
                        k += 1
            assert k <= 128, f"target cap exceeded: {k}"
            percore[c][f"tsrc{bi}"] = offs
            percore[c][f"tslot{bi}"] = wrap16(slots, 8)
            o = np.zeros((128, 1), np.int32)
            for tset in range(2):
                for j in range(GPC):
                    o[tset * GPC + j, 0] = bi * 512 + tset * 256 + c * GPC + j
            percore[c][f"koff{bi}"] = o

    shared = {}
    for bi, p in ((0, "1"), (1, "2")):
        w1 = np.asarray(inputs[f"mlp{p}_w1"])
        w2 = np.asarray(inputs[f"mlp{p}_w2"])
        for l in range(NUM):
            shared[f"w1_{bi}_{l}"] = w1[l].astype(BF)
            shared[f"w2_{bi}_{l}"] = w2[l].astype(BF)
        shared[f"b1_{bi}"] = np.asarray(inputs[f"mlp{p}_b1"]).T.copy()
        shared[f"b2_{bi}"] = np.asarray(inputs[f"mlp{p}_b2"]).T.copy()
        shared[f"gam_{bi}"] = np.asarray(inputs[f"bn{p}_gamma"]).T.copy()
        shared[f"bet_{bi}"] = np.asarray(inputs[f"bn{p}_beta"]).T.copy()
        for nm in ("Wq", "Wk", "Wv"):
            shared[f"{nm}_{bi}"] = np.ascontiguousarray(
                np.asarray(inputs[f"{nm}{p}"]).transpose(1, 0, 2)
                .reshape(DIM + 1, DIM)).astype(BF)
    shared["fc11_w"] = np.asarray(inputs["fc11_w"])
    shared["fc11_b"] = np.asarray(inputs["fc11_b"])[:, None]
    shared["fc12_w"] = np.asarray(inputs["fc12_w"])
    shared["fc12_b"] = np.asarray(inputs["fc12_b"])[:, None]
    shared["bd8"] = (np.kron(np.eye(HEAD, dtype=np.float32),
                             np.ones((DH, 1), np.float32)) / np.sqrt(DH)
                     ).astype(BF)
    shared["iota"] = np.broadcast_to(
        np.arange(128, dtype=np.float32), (128, 128)).astype(BF).copy()
    for c in range(NCORES):
        percore[c].update(shared)
    return sched, percore


def bap(t_ap, offset_elems, ap_list):
    return bass.AP(t_ap.tensor, t_ap.offset + offset_elems, ap_list)


def build(sched):
    NT, NW, NCH = sched["NT"], sched["NW"], sched["NCH"]
    NXC = NT // 128
    nc = bacc.Bacc(None, target_bir_lowering=False)

    def P(name, shape, dt=F32):
        return nc.declare_dram_parameter(name, list(shape), dt, isOutput=False)

    NB = [sched["NB0"], sched["NB1"]]
    ins = {}
    for bi in range(2):
        ins[f"gidx{bi}"] = P(f"gidx{bi}", [128, NB[bi] * 8], I16)
        ins[f"rel{bi}"] = P(f"rel{bi}", [128, NB[bi]], BF16)
        ins[f"xidx{bi}"] = P(f"xidx{bi}", [128, NT // 16], I16)
        ins[f"embt{bi}"] = P(f"embt{bi}", [128, DIM])
        ins[f"tsrc{bi}"] = P(f"tsrc{bi}", [128, 1], I32)
        ins[f"tslot{bi}"] = P(f"tslot{bi}", [128, 8], I16)
        ins[f"koff{bi}"] = P(f"koff{bi}", [128, 1], I32)
        for nm in ("Wq", "Wk", "Wv"):
            ins[f"{nm}_{bi}"] = P(f"{nm}_{bi}", [DIM + 1, DIM], BF16)
        for l in range(NUM):
            ins[f"w1_{bi}_{l}"] = P(f"w1_{bi}_{l}", [DIM, DIM], BF16)
            ins[f"w2_{bi}_{l}"] = P(f"w2_{bi}_{l}", [DIM, DIM], BF16)
        for nm in ("b1", "b2", "gam", "bet"):
            ins[f"{nm}_{bi}"] = P(f"{nm}_{bi}", [DIM, NUM])
    ins["attr"] = P("attr", [128, NB[0] * DIM], BF16)
    ins["mask"] = P("mask", [64, NT], BF16)
    ins["G"] = P("G", [GPC, NT], BF16)
    ins["GT"] = P("GT", [128, NXC * GPC], BF16)
    ins["dirn"] = P("dirn", [1, NT], BF16)
    ins["dirt"] = P("dirt", [1, 2 * GPC], BF16)
    ins["dir2"] = P("dir2", [1, GPC])
    ins["fc11_w"] = P("fc11_w", [DIM + 1, 32])
    ins["fc11_b"] = P("fc11_b", [32, 1])
    ins["fc12_w"] = P("fc12_w", [32, 1])
    ins["fc12_b"] = P("fc12_b", [1, 1])
    ins["bd8"] = P("bd8", [64, HEAD], BF16)
    ins["iota"] = P("iota", [128, 128], BF16)

    out_ext = nc.declare_dram_parameter("out", [1, GPC], F32, isOutput=True)

    hb = [nc.dram_tensor(f"h{bi}b", [NT, DIM], F32) for bi in range(2)]
    hTd = [nc.dram_tensor(f"h{bi}Td", [64, NT], F32) for bi in range(2)]
    table = [nc.dram_tensor(f"table{bi}", [NCORES * NT, DIM], F32,
                            addr_space="Shared") for bi in range(2)]
    stats_in = [nc.dram_tensor(f"stats_in{bi}", [64, 2], F32)
                for bi in range(2)]
    stats_out = [nc.dram_tensor(f"stats_out{bi}", [64, 2], F32,
                                addr_space="Shared") for bi in range(2)]
    TROWS = 1024 + 16
    tb_in = nc.dram_tensor("tb_in", [TROWS, DIM], F32)
    tb_out = nc.dram_tensor("tb_out", [TROWS, DIM], F32, addr_space="Shared")
    hT65d = [nc.dram_tensor(f"h{bi}T65d", [DIM + 1, NT], BF16)
             for bi in range(2)]
    RG = [list(range(NCORES))]

    secchunk = [sched["secchunk0"], sched["secchunk1"]]
    CWb = [sched["CW0"], sched["CW1"]]
    lsec = [sched["lsec0"], sched["lsec1"]]
    cell_off = [sched["cell_off0"], sched["cell_off1"]]
    nmc = NT // 512

    with tile.TileContext(nc) as tc:
        nc.gpsimd.load_library(library_config.mlp)
        with (
            tc.tile_pool(name="res", bufs=1) as res,
            tc.tile_pool(name="big", bufs=2) as big,
            tc.tile_pool(name="selp", bufs=1) as selp,
            tc.tile_pool(name="sm", bufs=1) as sm,
            tc.tile_pool(name="smd", bufs=2) as smd,
            tc.tile_pool(name="ps", bufs=2, space="PSUM") as ps,
            tc.tile_pool(name="pse", bufs=1, space="PSUM") as pse,
        ):
            ident = res.tile([128, 128], F32, tag="ident")
            make_identity(nc, ident[:])
            iota_t = res.tile([128, 128], BF16, tag="iota")
            nc.sync.dma_start(iota_t[:], ins["iota"][:])
            rel_t = [res.tile([128, NB[bi]], BF16, tag=f"rel{bi}",
                              name=f"rel_t{bi}") for bi in range(2)]
            for bi in range(2):
                nc.sync.dma_start(rel_t[bi][:], ins[f"rel{bi}"][:])

            xT = res.tile([64, NT], BF16, tag="xT")
            tcache = [res.tile([64, NT], BF16, tag=f"tc{bi}",
                               name=f"tcache{bi}") for bi in range(2)]
            # e values for both target-sets, interleaved: col x*16 + tset*8 + h
            eT = res.tile([128, NXC * 2 * HEAD], BF16, tag="eT")
            GTt = res.tile([128, NXC * GPC], BF16, tag="GT")
            nc.sync.dma_start(GTt[:], ins["GT"][:])
            osum = res.tile([64, GPC], F32, tag="osum")

            zt = sm.tile([128, 512], F32, tag="zt")
            nc.vector.memset(zt[:], 0.0)
            for bi in range(2):
                flat = hb[bi].ap().rearrange("(a b) e -> a (b e)", a=128)
                wtot = flat.shape[1]
                o = 0
                while o < wtot:
                    wd = min(512, wtot - o)
                    nc.sync.dma_start(flat[:, o:o + wd], zt[:, :wd])
                    o += wd

            # ================= embeddings =================
            for bi in range(2):
                xi = sm.tile([128, NT // 16], I16, tag="xi")
                nc.sync.dma_start(xi[:], ins[f"xidx{bi}"][:])
                QH = NXC // 4
                for q in range(4):
                    h0c = big.tile([128, QH * DIM], F32, tag="gt")
                    h03 = h0c[:].rearrange("p (n e) -> p n e", e=DIM)
                    nc.gpsimd.dma_gather(
                        h03, ins[f"embt{bi}"][:],
                        xi[:, q * (QH * 8):(q + 1) * (QH * 8)],
                        QH * 128, QH * 128, DIM, single_packet=False)
                    nc.sync.dma_start(
                        hb[bi].ap().rearrange("(n p) e -> p n e", p=128)
                        [:, q * QH:(q + 1) * QH, :], h03)
                    for xx in range(QH):
                        x = q * QH + xx
                        pt = pse.tile([64, 128], F32, tag="misc",
                                      name=f"tp_{bi}_{x}")
                        nc.tensor.transpose(pt[:],
                                            h0c[:, xx * DIM:(xx + 1) * DIM],
                                            ident[:])
                        hx = smd.tile([64, 128], F32, tag="hx")
                        nc.scalar.activation(hx[:], pt[:], AF.Copy)
                        nc.sync.dma_start(
                            hTd[bi].ap()[:, x * 128:(x + 1) * 128], hx[:])
                nc.gpsimd.collective_compute(
                    "AllGather", ALU.bypass, replica_groups=RG,
                    ins=[hb[bi].ap().opt()], outs=[table[bi].ap().opt()])

            wt = {}
            for bi in range(2):
                for l in range(NUM):
                    for nm in ("w1", "w2"):
                        t = res.tile([DIM, DIM], BF16, tag=f"{nm}{bi}{l}",
                                     name=f"{nm}_{bi}_{l}t")
                        nc.sync.dma_start(t[:], ins[f"{nm}_{bi}_{l}"][:])
                        wt[f"{nm}{bi}{l}"] = t
                for nm in ("b1", "b2", "gam", "bet"):
                    t = res.tile([DIM, NUM], F32, tag=f"{nm}{bi}",
                                 name=f"{nm}_{bi}t")
                    nc.sync.dma_start(t[:], ins[f"{nm}_{bi}"][:])
                    wt[f"{nm}{bi}"] = t

            # ================= main layers =================
            for l in range(NUM):
                for bi in range(2):
                    # ---- scatter: xT = agg + h ----
                    for ch in range(NCH):
                        w_lo, w_hi = ch * CHW, min((ch + 1) * CHW, NW)
                        hc = smd.tile([64, CHW * 128], F32, tag="hc")
                        nc.sync.dma_start(hc[:, :(w_hi - w_lo) * 128],
                                          hTd[bi].ap()[:, w_lo * 128:w_hi * 128])
                        msgs = {}
                        for s_ in range(NSEC):
                            b0 = int(secchunk[bi][s_, ch])
                            b1_ = int(secchunk[bi][s_, ch + 1])
                            nb = b1_ - b0
                            if nb == 0:
                                continue
                            gi = smd.tile([128, nb * 8], I16, tag="gi")
                            nc.sync.dma_start(gi[:],
                                              ins[f"gidx{bi}"][:, b0 * 8:b1_ * 8])
                            gt = big.tile([128, nb * DIM], F32, tag="gt")
                            gt3 = gt[:].rearrange("p (n e) -> p n e", e=DIM)
                            nc.gpsimd.dma_gather(
                                gt3, table[bi][s_ * 2 * NT:(s_ + 1) * 2 * NT, :],
                                gi[:], nb * 128, nb * 128, DIM,
                                single_packet=False)
                            msg = big.tile([128, nb * DIM], BF16, bufs=1,
                                           tag=f"msg{s_}", name=f"msg{s_}_t")
                            if bi == 0:
                                at = big.tile([128, nb * DIM], BF16, tag="at")
                                nc.sync.dma_start(
                                    at[:], ins["attr"][:, b0 * DIM:b1_ * DIM])
                                nc.vector.tensor_tensor(
                                    out=msg[:], in0=gt[:], in1=at[:], op=ALU.add)
                                nc.vector.tensor_scalar_max(msg[:], msg[:], 0.0)
                            else:
                                nc.vector.tensor_copy(msg[:], gt[:])
                            msgs[s_] = (msg, b0)
                        for w in range(w_lo, w_hi):
                            sl = slice(w * 128, (w + 1) * 128)
                            hsl = slice((w - w_lo) * 128, (w - w_lo + 1) * 128)
                            if CWb[bi][:, w].sum() == 0:
                                nc.vector.tensor_copy(xT[:, sl], hc[:, hsl])
                                continue
                            pw = ps.tile([64, 128], F32, tag="pw", bufs=2,
                                         name=f"pw_{l}_{bi}_{w}")
                            started = False
                            for s_ in range(NSEC):
                                cw = int(CWb[bi][s_, w])
                                if cw == 0:
                                    continue
                                msg, b0 = msgs[s_]
                                msg3 = msg[:].rearrange("p (n e) -> p n e", e=DIM)
                                c0 = int(cell_off[bi][s_, w])
                                sel = selp.tile([128, cw * 128], BF16,
                                                tag="sel", bufs=2)
                                sel3 = sel[:].rearrange("p (n e) -> p n e", e=128)
                                io_ap = bap(iota_t[:], 0,
                                            [[128, 128], [0, cw], [1, 128]])
                                rl_ap = bap(rel_t[bi][:], c0,
                                            [[NB[bi], 128], [1, cw], [0, 128]])
                                nc.vector.tensor_tensor(
                                    out=sel3[:], in0=io_ap, in1=rl_ap,
                                    op=ALU.is_equal)
                                for j in range(cw):
                                    nc.tensor.matmul(
                                        pw[:], lhsT=msg3[:, c0 - b0 + j, :],
                                        rhs=sel3[:, j, :],
                                        start=(not started),
                                        stop=(s_ == lsec[bi][w] and j == cw - 1))
                                    started = True
                            nc.vector.tensor_tensor(
                                out=xT[:, sl], in0=pw[:], in1=hc[:, hsl],
                                op=ALU.add)

                    # ---- MLP -> tcache (masked, bf16) + stats ----
                    b1c = wt[f"b1{bi}"][:, l:l + 1]
                    b2c = wt[f"b2{bi}"][:, l:l + 1]
                    parts = sm.tile([64, 2 * nmc], F32, tag=f"parts{bi}",
                                    name=f"parts{bi}")
                    for jc in range(nmc):
                        j0 = jc * 512
                        p1 = ps.tile([64, 512], F32, tag="p1", bufs=1)
                        nc.tensor.matmul(p1[:], lhsT=wt[f"w1{bi}{l}"][:],
                                         rhs=xT[:, j0:j0 + 512], start=True,
                                         stop=True)
                        t1 = smd.tile([64, 512], BF16, tag="t1")
                        nc.scalar.activation(t1[:], p1[:], AF.Relu, bias=b1c)
                        p2 = ps.tile([64, 512], F32, tag="p2", bufs=1)
                        nc.tensor.matmul(p2[:], lhsT=wt[f"w2{bi}{l}"][:],
                                         rhs=t1[:], start=True, stop=True)
                        tr = smd.tile([64, 512], F32, tag="tr")
                        nc.scalar.activation(tr[:], p2[:], AF.Identity, bias=b2c)
                        mk = smd.tile([64, 512], BF16, tag="mk")
                        nc.sync.dma_start(mk[:], ins["mask"][:, j0:j0 + 512])
                        tm = smd.tile([64, 512], F32, tag="tm")
                        nc.vector.tensor_tensor(out=tm[:], in0=tr[:],
                                                in1=mk[:], op=ALU.mult)
                        nc.vector.tensor_copy(tcache[bi][:, j0:j0 + 512], tm[:])
                        sq = smd.tile([64, 512], F32, tag="sq")
                        nc.vector.tensor_tensor(out=sq[:], in0=tm[:], in1=tm[:],
                                                op=ALU.mult)
                        nc.vector.reduce_sum(parts[:, jc:jc + 1], tm[:],
                                             axis=mybir.AxisListType.X)
                        nc.vector.reduce_sum(parts[:, nmc + jc:nmc + jc + 1],
                                             sq[:], axis=mybir.AxisListType.X)
                    st = sm.tile([64, 2], F32, tag=f"st{bi}", name=f"st{bi}")
                    nc.vector.reduce_sum(st[:, 0:1], parts[:, :nmc],
                                         axis=mybir.AxisListType.X)
                    nc.vector.reduce_sum(st[:, 1:2], parts[:, nmc:2 * nmc],
                                         axis=mybir.AxisListType.X)
                    nc.sync.dma_start(stats_in[bi].ap()[:, :], st[:])
                    nc.gpsimd.collective_compute(
                        "AllReduce", ALU.add, replica_groups=RG,
                        ins=[stats_in[bi].ap().opt()],
                        outs=[stats_out[bi].ap().opt()])

                for bi in range(2):
                    # ---- BN apply: h += gam*(t-m)/sig + bet ----
                    sts = sm.tile([64, 2], F32, tag=f"sts{bi}", name=f"sts{bi}")
                    nc.sync.dma_start(sts[:], stats_out[bi].ap()[:, :])
                    mv = sm.tile([64, 8], F32, tag=f"mv{bi}", name=f"mv{bi}")
                    nc.vector.tensor_scalar_mul(mv[:, 0:1], sts[:, 0:1], 1.0 / N)
                    nc.vector.tensor_scalar_mul(mv[:, 1:2], sts[:, 1:2], 1.0 / N)
                    nc.vector.tensor_tensor(out=mv[:, 2:3], in0=mv[:, 0:1],
                                            in1=mv[:, 0:1], op=ALU.mult)
                    nc.vector.tensor_tensor(out=mv[:, 3:4], in0=mv[:, 1:2],
                                            in1=mv[:, 2:3], op=ALU.subtract)
                    nc.vector.tensor_scalar_add(mv[:, 3:4], mv[:, 3:4], 1e-5)
                    nc.scalar.activation(mv[:, 4:5], mv[:, 3:4], AF.Sqrt)
                    nc.vector.reciprocal(mv[:, 5:6], mv[:, 4:5])
                    nc.vector.tensor_tensor(out=mv[:, 6:7],
                                            in0=wt[f"gam{bi}"][:, l:l + 1],
                                            in1=mv[:, 5:6], op=ALU.mult)
                    nc.vector.tensor_tensor(out=mv[:, 7:8], in0=mv[:, 0:1],
                                            in1=mv[:, 6:7], op=ALU.mult)
                    nc.vector.tensor_tensor(out=mv[:, 7:8],
                                            in0=wt[f"bet{bi}"][:, l:l + 1],
                                            in1=mv[:, 7:8], op=ALU.subtract)
                    for jc in range(nmc):
                        j0 = jc * 512
                        hc2 = smd.tile([64, 512], F32, tag="hc2")
                        nc.sync.dma_start(hc2[:], hTd[bi].ap()[:, j0:j0 + 512])
                        sc1 = smd.tile([64, 512], F32, tag="sc1")
                        nc.scalar.activation(sc1[:], tcache[bi][:, j0:j0 + 512],
                                             AF.Identity, scale=mv[:, 6:7],
                                             bias=mv[:, 7:8])
                        nc.vector.tensor_tensor(out=hc2[:], in0=hc2[:],
                                                in1=sc1[:], op=ALU.add)
                        nc.sync.dma_start(hTd[bi].ap()[:, j0:j0 + 512], hc2[:])
                        hnm4 = smd.tile([128, 4 * DIM], F32, tag="hnm4")
                        for xx in range(4):
                            pt2 = pse.tile([128, 64], F32, tag="misc",
                                           name=f"tp2_{l}_{bi}_{jc}_{xx}")
                            nc.tensor.transpose(
                                pt2[:], hc2[:, xx * 128:(xx + 1) * 128],
                                ident[:64, :64])
                            nc.scalar.activation(
                                hnm4[:, xx * DIM:(xx + 1) * DIM], pt2[:],
                                AF.Copy)
                        nc.sync.dma_start(
                            hb[bi].ap().rearrange("(n p) e -> p n e", p=128)
                            [:, jc * 4:jc * 4 + 4, :],
                            hnm4[:].rearrange("p (n e) -> p n e", e=DIM))
                    if l < NUM - 1:
                        nc.gpsimd.collective_compute(
                            "AllGather", ALU.bypass, replica_groups=RG,
                            ins=[hb[bi].ap().opt()], outs=[table[bi].ap().opt()])

            # ================= hT65 (final h, bf16, + direction row) ========
            for bi in range(2):
                for jc in range(nmc):
                    j0 = jc * 512
                    hcf = smd.tile([64, 512], F32, tag="hcf")
                    nc.sync.dma_start(hcf[:], hTd[bi].ap()[:, j0:j0 + 512])
                    hcb = smd.tile([64, 512], BF16, tag="hcb")
                    nc.vector.tensor_copy(hcb[:], hcf[:])
                    nc.sync.dma_start(hT65d[bi].ap()[:64, j0:j0 + 512], hcb[:])
                nc.sync.dma_start(hT65d[bi].ap()[64:65, :], ins["dirn"][:])

            # ================= target rows =================
            for r0 in range(0, TROWS, 128):
                rr = min(128, TROWS - r0)
                nc.sync.dma_start(tb_in.ap()[r0:r0 + rr, :], zt[:rr, :DIM])
            for bi in range(2):
                toff = sm.tile([128, 1], I32, tag="toff", name=f"toff{bi}")
                nc.sync.dma_start(toff[:], ins[f"tsrc{bi}"][:])
                trows = sm.tile([128, DIM], F32, tag="trows", name=f"trows{bi}")
                nc.gpsimd.indirect_dma_start(
                    out=trows[:], out_offset=None, in_=hb[bi].ap(),
                    in_offset=bass.IndirectOffsetOnAxis(ap=toff[:, 0:1], axis=0))
                tsl = sm.tile([128, 8], I16, tag="tsl", name=f"tsl{bi}")
                nc.sync.dma_start(tsl[:], ins[f"tslot{bi}"][:])
                nc.gpsimd.dma_scatter_add(
                    tb_in.ap()[:, :],
                    trows[:].rearrange("p (n e) -> p n e", e=DIM),
                    tsl[:], 128, 128, DIM, single_packet=False)
            nc.gpsimd.collective_compute(
                "AllReduce", ALU.add, replica_groups=RG,
                ins=[tb_in.ap().opt()], outs=[tb_out.ap().opt()])

            # ================= attention =================
            bd = sm.tile([64, HEAD], BF16, tag="bd")
            nc.sync.dma_start(bd[:], ins["bd8"][:])

            for bi in range(2):
                wq65 = sm.tile([DIM + 1, DIM], BF16, tag="wq", name=f"wq{bi}")
                nc.sync.dma_start(wq65[:], ins[f"Wq_{bi}"][:])
                wk65 = sm.tile([DIM + 1, DIM], BF16, tag="wk", name=f"wk{bi}")
                nc.sync.dma_start(wk65[:], ins[f"Wk_{bi}"][:])
                wv65 = sm.tile([DIM + 1, DIM], BF16, tag="wv", name=f"wv{bi}")
                nc.sync.dma_start(wv65[:], ins[f"Wv_{bi}"][:])
                ko = sm.tile([128, 1], I32, tag="ko", name=f"ko{bi}")
                nc.sync.dma_start(ko[:], ins[f"koff{bi}"][:])
                krows = sm.tile([128, DIM], F32, tag="krows", name=f"krows{bi}")
                nc.gpsimd.indirect_dma_start(
                    out=krows[:], out_offset=None, in_=tb_out.ap(),
                    in_offset=bass.IndirectOffsetOnAxis(ap=ko[:, 0:1], axis=0))
                kp = pse.tile([64, 128], F32, tag="misc", name=f"kp{bi}")
                nc.tensor.transpose(kp[:], krows[:], ident[:])
                kft65 = sm.tile([DIM + 1, 2 * GPC], BF16, tag="kft",
                                name=f"kft{bi}")
                nc.vector.tensor_copy(kft65[:64, :], kp[:, :2 * GPC])
                nc.sync.dma_start(kft65[64:65, :], ins["dirt"][:])
                ksp = pse.tile([64, 2 * GPC], F32, tag="misc2", name=f"ksp{bi}")
                nc.tensor.matmul(ksp[:], lhsT=wk65[:], rhs=kft65[:],
                                 start=True, stop=True)
                kT = sm.tile([64, 2 * GPC], F32, tag="kT", name=f"kT{bi}")
                nc.vector.tensor_copy(kT[:], ksp[:])
                knms = []
                for tset in range(2):
                    knm_p = pse.tile([GPC, 64], F32, tag="misc",
                                     name=f"knmp_{bi}_{tset}")
                    nc.tensor.transpose(knm_p[:],
                                        kT[:, tset * GPC:(tset + 1) * GPC],
                                        ident[:64, :64])
                    knm1 = sm.tile([GPC, 64], BF16, tag=f"knmb{tset}",
                                   name=f"knm_{bi}_{tset}")
                    nc.vector.tensor_copy(knm1[:], knm_p[:])
                    knms.append(knm1)

                # pass 1: scores -> eT (both tsets), denominators dp
                dps = [pse.tile([GPC, HEAD], F32, tag=f"acc{'2' if t else ''}",
                                name=f"dp_{bi}_{t}") for t in range(2)]
                for jc in range(nmc):
                    j0 = jc * 512
                    sl = slice(j0, j0 + 512)
                    h65c = smd.tile([DIM + 1, 512], BF16, tag="h65c")
                    nc.sync.dma_start(h65c[:], hT65d[bi].ap()[:, sl])
                    gch = smd.tile([GPC, 512], BF16, tag="gch")
                    nc.sync.dma_start(gch[:], ins["G"][:, sl])
                    qp = ps.tile([64, 512], F32, tag="p1", bufs=1,
                                 name=f"qp_{bi}_{jc}")
                    nc.tensor.matmul(qp[:], lhsT=wq65[:], rhs=h65c[:],
                                     start=True, stop=True)
                    for tset in range(2):
                        kgp = ps.tile([64, 512], F32, tag="p2", bufs=1,
                                      name=f"kgp_{bi}_{jc}_{tset}")
                        nc.tensor.matmul(kgp[:], lhsT=knms[tset][:],
                                         rhs=gch[:], start=True, stop=True)
                        kgs = smd.tile([64, 512], F32, tag="kgs")
                        nc.vector.tensor_copy(kgs[:], kgp[:])
                        qk = smd.tile([64, 512], BF16, tag="qk")
                        nc.vector.tensor_tensor(out=qk[:], in0=qp[:],
                                                in1=kgs[:], op=ALU.mult)
                        for xx in range(4):
                            x = jc * 4 + xx
                            ec = (2 * x + tset) * HEAD
                            sT = pse.tile([128, HEAD], F32, tag="misc2",
                                          name=f"sT_{bi}_{tset}_{x}")
                            nc.tensor.matmul(
                                sT[:], lhsT=qk[:, xx * 128:(xx + 1) * 128],
                                rhs=bd[:], start=True, stop=True)
                            nc.scalar.activation(
                                eT[:, ec:ec + HEAD], sT[:], AF.Exp)
                            nc.tensor.matmul(
                                dps[tset][:],
                                lhsT=GTt[:, x * GPC:(x + 1) * GPC],
                                rhs=eT[:, ec:ec + HEAD],
                                start=(x == 0), stop=(x == NXC - 1))
                dt = sm.tile([GPC, 2 * HEAD], F32, tag="d0", name=f"dt_{bi}")
                nc.vector.tensor_copy(dt[:, :HEAD], dps[0][:])
                nc.vector.tensor_copy(dt[:, HEAD:], dps[1][:])
                nc.vector.tensor_scalar_add(dt[:], dt[:], 1e-16)
                nc.vector.reciprocal(dt[:], dt[:])
                dtb = sm.tile([GPC, 2 * HEAD], BF16, tag="db0",
                              name=f"dtb_{bi}")
                nc.vector.tensor_copy(dtb[:], dt[:])

                # pass 2: weighted V accumulation (node-major)
                op_ = pse.tile([64, GPC], F32, tag="acc2", name=f"op{bi}")
                for jc in range(nmc):
                    j0 = jc * 512
                    h65c = smd.tile([DIM + 1, 512], BF16, tag="h65c")
                    nc.sync.dma_start(h65c[:], hT65d[bi].ap()[:, j0:j0 + 512])
                    gch = smd.tile([GPC, 512], BF16, tag="gch")
                    nc.sync.dma_start(gch[:], ins["G"][:, j0:j0 + 512])
                    for xx in range(4):
                        x = jc * 4 + xx
                        xsl = slice(xx * 128, (xx + 1) * 128)
                        rg = pse.tile([128, 2 * HEAD], F32, tag="misc",
                                      name=f"rg_{bi}_{x}")
                        nc.tensor.matmul(rg[:], lhsT=gch[:, xsl], rhs=dtb[:],
                                         start=True, stop=True)
                        wboth = smd.tile([128, 2 * HEAD], BF16, tag="wboth")
                        nc.vector.tensor_tensor(
                            out=wboth[:], in0=rg[:],
                            in1=eT[:, 2 * x * HEAD:2 * (x + 1) * HEAD],
                            op=ALU.mult)
                        wsum = smd.tile([128, HEAD], BF16, tag="wsum")
                        nc.vector.tensor_tensor(out=wsum[:],
                                                in0=wboth[:, :HEAD],
                                                in1=wboth[:, HEAD:],
                                                op=ALU.add)
                        vp = pse.tile([128, 64], F32, tag="misc2",
                                      name=f"vp_{bi}_{x}")
                        nc.tensor.matmul(vp[:], lhsT=h65c[:, xsl], rhs=wv65[:],
                                         start=True, stop=True)
                        wvn = smd.tile([128, 64], BF16, tag="wvn")
                        wb_ap = bap(wsum[:], 0,
                                    [[HEAD, 128], [1, HEAD], [0, DH]])
                        nc.vector.tensor_tensor(
                            out=wvn[:].rearrange("p (h r) -> p h r", r=DH),
                            in0=vp[:].rearrange("p (h r) -> p h r", r=DH),
                            in1=wb_ap, op=ALU.mult)
                        nc.tensor.matmul(op_[:], lhsT=wvn[:],
                                         rhs=GTt[:, x * GPC:(x + 1) * GPC],
                                         start=(x == 0), stop=(x == NXC - 1))
                if bi == 0:
                    nc.vector.tensor_copy(osum[:], op_[:])
                else:
                    nc.vector.tensor_tensor(out=osum[:], in0=osum[:],
                                            in1=op_[:], op=ALU.add)

            fw1 = sm.tile([DIM, 32], F32, tag="fw1")
            nc.sync.dma_start(fw1[:], ins["fc11_w"][:DIM, :])
            fw1r = sm.tile([1, 32], F32, tag="fw1r")
            nc.sync.dma_start(fw1r[:], ins["fc11_w"][DIM:DIM + 1, :])
            fb1 = sm.tile([32, 1], F32, tag="fb1")
            nc.sync.dma_start(fb1[:], ins["fc11_b"][:])
            fw2 = sm.tile([32, 1], F32, tag="fw2")
            nc.sync.dma_start(fw2[:], ins["fc12_w"][:])
            fb2 = sm.tile([1, 1], F32, tag="fb2")
            nc.sync.dma_start(fb2[:], ins["fc12_b"][:])
            dir2_t = sm.tile([1, GPC], F32, tag="dir2")
            nc.sync.dma_start(dir2_t[:], ins["dir2"][:])
            zp = pse.tile([32, GPC], F32, tag="misc", name="zp_t")
            nc.tensor.matmul(zp[:], lhsT=fw1[:], rhs=osum[:], start=True,
                             stop=False)
            nc.tensor.matmul(zp[:], lhsT=fw1r[:], rhs=dir2_t[:], start=False,
                             stop=True)
            zs = sm.tile([32, GPC], F32, tag="zs")
            nc.scalar.activation(zs[:], zp[:], AF.Gelu, bias=fb1[:, 0:1])
            op2 = pse.tile([1, GPC], F32, tag="misc2", name="op2_t")
            nc.tensor.matmul(op2[:], lhsT=fw2[:], rhs=zs[:], start=True,
                             stop=True)
            fin = sm.tile([1, GPC], F32, tag="fin")
            nc.scalar.activation(fin[:], op2[:], AF.Sigmoid, bias=fb2[:, 0:1])
            nc.sync.dma_start(out_ext.ap()[:, :], fin[:])

    nc.compile()
    return nc


def kernel(**inputs):
    sched, percore = preprocess(inputs)
    nc = build(sched)
    res = run_bass_kernel_spmd(nc, percore, list(range(NCORES)))
    outs = [res.results[c]["out"][0] for c in range(NCORES)]
    return np.concatenate(outs).astype(np.float32)[:, None]


def kernel_timed(n_iter=8, **inputs):
    """kernel() plus a wall-clock estimate of per-NEFF execution time using
    pipelined repeated executions of the compiled executable."""
    import time
    import jax
    from jax.sharding import Mesh, PartitionSpec
    from jax.experimental.shard_map import shard_map
    from concourse import bass2jax

    sched, percore = preprocess(inputs)
    nc = build(sched)

    bass2jax.install_neuronx_cc_hook()
    partition_name = nc.partition_id_tensor.name if nc.partition_id_tensor else None
    in_names, out_names, out_avals, zero_outs = [], [], [], []
    for alloc in nc.m.functions[0].allocations:
        if not isinstance(alloc, mybir.MemoryLocationSet):
            continue
        name = alloc.memorylocations[0].name
        if alloc.kind == "ExternalInput":
            if name != partition_name:
                in_names.append(name)
        elif alloc.kind == "ExternalOutput":
            out_names.append(name)
            shape = tuple(alloc.tensor_shape)
            dtype = mybir.dt.np(alloc.dtype)
            out_avals.append(jax.core.ShapedArray(shape, dtype))
            zero_outs.append(np.zeros(shape, dtype))
    n_params = len(in_names)
    n_outs = len(out_avals)
    in_names.extend(out_names)
    if partition_name is not None:
        in_names.append(partition_name)

    def _body(*args):
        operands = list(args)
        if partition_name is not None:
            operands.append(bass2jax.partition_id_tensor())
        outs = bass2jax._bass_exec_p.bind(
            *operands, out_avals=tuple(out_avals), in_names=tuple(in_names),
            out_names=tuple(out_names), lowering_input_output_aliases=(),
            sim_require_finite=True, sim_require_nnan=True, nc=nc)
        return tuple(outs)

    devices = jax.devices()[:NCORES]
    mesh = Mesh(np.asarray(devices), ("core",))
    in_specs = (PartitionSpec("core"),) * (n_params + n_outs)
    out_specs = (PartitionSpec("core"),) * len(out_names)
    sharded = jax.jit(shard_map(_body, mesh=mesh, in_specs=in_specs,
                                out_specs=out_specs, check_rep=False),
                      keep_unused=True)
    per_core = [[np.asarray(percore[c][name]) for name in in_names[:n_params]]
                for c in range(NCORES)]
    concat_in = [np.concatenate([per_core[c][i] for c in range(NCORES)], axis=0)
                 for i in range(n_params)]
    concat_zeros = [np.zeros((NCORES * z.shape[0], *z.shape[1:]), z.dtype)
                    for z in zero_outs]
    args = [jax.device_put(a) for a in concat_in + concat_zeros]
    out = sharded(*args)
    jax.block_until_ready(out)
    # warm single call for dispatch-overhead baseline, then pipelined batch
    t0 = time.perf_counter()
    outs = [sharded(*args) for _ in range(n_iter)]
    jax.block_until_ready(outs)
    per_call = (time.perf_counter() - t0) / n_iter
    oidx = out_names.index("out")
    res = np.asarray(outs[-1][oidx]).reshape(NCORES, 1, GPC)
    full = np.concatenate([res[c][0] for c in range(NCORES)]).astype(np.float32)[:, None]
    return full, per_call
